# revision 16
# baseline (speedup 1.0000x reference)
"""Block-sparse linear (x @ W.T + bias) on 8 TRN2 NeuronCores.

Data-parallel over tokens: each core computes all 4096 output features
for its 1024-token shard.  The sparse blocks are scattered into a dense
W on the host; the device runs a dense bf16 matmul (fp32 PSUM
accumulation — fp32 matmul is 4 cycles/row on the PE, bf16 is 1):

  * x shard host-converted to bf16 and transposed to SBUF layout
    [128 part, 32 kchunk, 1024 tok] (in-features on partitions),
    resident in SBUF (8 MB).
  * W.T streamed as 32 output-tiles of [128 in, 32 kchunk, 128 out]
    (1 MB each, multi-buffered DMA prefetch).
  * Per (o-tile, token half): one PSUM bank accumulates 32 K-chunk
    matmuls (K=128, M=128, N=512, bf16 in / fp32 acc), then DVE-drains
    to SBUF and DMAs to the transposed output outT [4096, 1024] fp32.

Host concatenates/transposes per-core outT shards and adds bias.
"""

import os
import sys
import time
import types
from contextlib import ExitStack

import ml_dtypes
import numpy as np

import concourse.tile as tile
from concourse import bacc, mybir
from concourse.bass_utils import run_bass_kernel_spmd


def _install_ntff_hook_shim():
    """Provide antenv.axon_hooks if the image lacks it, so BASS_TRACE=1
    can capture NTFF profiles through libaxon_pjrt."""
    try:
        import antenv.axon_hooks  # noqa: F401

        return
    except ImportError:
        pass
    try:
        import antenv
    except ImportError:
        return
    mod = types.ModuleType("antenv.axon_hooks")
    holder = {"hook": None}
    mod.set_axon_ntff_profile_hook = lambda h: holder.__setitem__("hook", h)
    mod.get_axon_ntff_profile_hook = lambda: holder["hook"]
    sys.modules["antenv.axon_hooks"] = mod
    antenv.axon_hooks = mod
    try:
        from trn_agent_boot.trn_boot import _ntff_profile_via_ctypes

        so_path = "/opt/axon/libaxon_pjrt.so"
        if os.path.exists(so_path):
            hook = _ntff_profile_via_ctypes(so_path)
            if hook is not None:
                mod.set_axon_ntff_profile_hook(hook)
    except Exception:
        pass


_install_ntff_hook_shim()

BLOCK = 32
IN_FEATURES = 4096
OUT_FEATURES = 4096
N_TOKENS = 8192
N_CORES = 8
NBC = IN_FEATURES // BLOCK  # 128
TOK_PER_CORE = N_TOKENS // N_CORES  # 1024
MM_N = 512
NPHASE = TOK_PER_CORE // MM_N  # 2
KCH = IN_FEATURES // 128  # 32 contraction chunks
OT = OUT_FEATURES // 128  # 32 output tiles
F32 = mybir.dt.float32
BF16 = mybir.dt.bfloat16
NP_BF16 = np.dtype(ml_dtypes.bfloat16)

LAST_RUN = None  # BassKernelResults of the most recent kernel() call


def _build_program():
    nc = bacc.Bacc(trn_type="TRN2", target_bir_lowering=False, debug=False)
    # x pre-split by token phase on host: [phase, 128 part, kch, 512 tok]
    xTd = nc.dram_tensor("xT", [NPHASE, 128, KCH, MM_N], BF16, kind="ExternalInput").ap()
    wst = nc.dram_tensor("wst", [OT, 128, KCH, 128], BF16, kind="ExternalInput").ap()
    outT = nc.dram_tensor(
        "outT", [OUT_FEATURES, TOK_PER_CORE], F32, kind="ExternalOutput"
    ).ap()

    with tile.TileContext(nc) as tc, ExitStack() as ctx:
        xpool = ctx.enter_context(tc.tile_pool(name="xpool", bufs=1))
        wpool = ctx.enter_context(tc.tile_pool(name="wpool", bufs=3))
        spool = ctx.enter_context(tc.tile_pool(name="spool", bufs=4))
        ppool = ctx.enter_context(tc.tile_pool(name="ppool", bufs=8, space="PSUM"))

        # First W tile ahead of the x stream so the PE can start ASAP.
        # w1 comes AFTER the phase-0 x half: o0's chain end is gated on that
        # 4.2MB delivery, so nothing else may queue ahead of it.
        w01 = []
        w = wpool.tile([128, KCH, 128], BF16, name="wt0_0", tag="wt")
        nc.sync.dma_start(w[:], wst[0])
        w01.append(w)

        # Phase-major: only the 4.2MB phase-0 half of x gates the pipeline
        # start; the phase-1 half streams in during phase-0 compute (its
        # triggers are spread across early o-iterations below so they queue
        # behind the W prefetch, not ahead of it).
        xts = []
        for ph in range(NPHASE):
            xp = xpool.tile([128, KCH, MM_N], BF16, name=f"xt{ph}", tag=f"xt{ph}")
            if ph == 0:
                for i in range(8):
                    sl = slice(4 * i, 4 * i + 4)
                    nc.sync.dma_start(xp[:, sl, :], xTd[ph, :, sl, :])
            xts.append(xp)

        w = wpool.tile([128, KCH, 128], BF16, name="wt0_1", tag="wt")
        nc.sync.dma_start(w[:], wst[1])
        w01.append(w)

        for ph in range(NPHASE):
            xp = xts[ph]
            for o in range(OT):
                if ph == 0 and o < 2:
                    wt = w01[o]
                else:
                    wt = wpool.tile([128, KCH, 128], BF16, name=f"wt{ph}_{o}", tag="wt")
                    nc.sync.dma_start(wt[:], wst[o])
                if ph == 0 and 2 <= o < 10:
                    i = o - 2
                    sl = slice(4 * i, 4 * i + 4)
                    nc.sync.dma_start(xts[1][:, sl, :], xTd[1, :, sl, :])
                ps = ppool.tile([128, MM_N], F32, name=f"ps{ph}_{o}", tag="ps")
                for k in range(KCH):
                    nc.tensor.matmul(
                        ps[:],
                        lhsT=wt[:, k, :],
                        rhs=xp[:, k, :],
                        start=(k == 0),
                        stop=(k == KCH - 1),
                    )
                stage = spool.tile([128, MM_N], F32, name=f"st{ph}_{o}", tag="st")
                if ph == NPHASE - 1 and o == OT - 1:
                    # Last tile: split drain so the final copy/DMA overlap,
                    # shrinking the kernel tail.
                    h = MM_N // 2
                    for j in range(2):
                        nc.vector.tensor_copy(
                            out=stage[:, j * h : (j + 1) * h], in_=ps[:, j * h : (j + 1) * h]
                        )
                        nc.sync.dma_start(
                            outT[
                                128 * o : 128 * o + 128,
                                MM_N * ph + j * h : MM_N * ph + (j + 1) * h,
                            ],
                            stage[:, j * h : (j + 1) * h],
                        )
                else:
                    nc.vector.tensor_copy(out=stage[:], in_=ps[:])
                    nc.sync.dma_start(
                        outT[128 * o : 128 * o + 128, MM_N * ph : MM_N * (ph + 1)],
                        stage[:],
                    )
    nc.compile()
    return nc


def kernel(x, weight_blocks, bias, block_rows, block_cols):
    global LAST_RUN
    x = np.asarray(x, np.float32)
    weight_blocks = np.asarray(weight_blocks, np.float32)
    bias = np.asarray(bias, np.float32)
    block_rows = np.asarray(block_rows).astype(np.int64)
    block_cols = np.asarray(block_cols).astype(np.int64)

    t0 = time.time()
    # dense W from sparse blocks (vectorized scatter into block grid;
    # harness block positions are a permutation => no duplicates)
    W4 = np.zeros((OUT_FEATURES // BLOCK, NBC, BLOCK, BLOCK), np.float32)
    W4[block_rows, block_cols] = weight_blocks
    W = W4.transpose(0, 2, 1, 3).reshape(OUT_FEATURES, IN_FEATURES)
    # wst[o, kk, k, mm] = W[128o+mm, 128k+kk]
    wst = np.ascontiguousarray(
        W.reshape(OT, 128, KCH, 128).transpose(0, 3, 2, 1).astype(NP_BF16)
    )

    nc = _build_program()
    t1 = time.time()

    xbf = x.astype(NP_BF16)
    in_maps = []
    for core in range(N_CORES):
        xs = xbf[core * TOK_PER_CORE : (core + 1) * TOK_PER_CORE]
        # [ph, 128 part, kch, 512 tok]; element (ph,p,k,t) = x[512ph+t, 128k+p]
        xt = np.ascontiguousarray(
            xs.T.reshape(KCH, 128, NPHASE, MM_N).transpose(2, 1, 0, 3)
        )
        in_maps.append({"xT": xt, "wst": wst})
    t2 = time.time()
    print(f"[kernel] prep={t1 - t0:.1f}s shards={t2 - t1:.1f}s", flush=True)

    res = run_bass_kernel_spmd(nc, in_maps, list(range(N_CORES)))
    LAST_RUN = res
    print(f"[kernel] run={time.time() - t2:.1f}s", flush=True)

    out = np.concatenate([res.results[i]["outT"].T for i in range(N_CORES)], axis=0)
    out = out + bias[None, :]
    return np.ascontiguousarray(out, np.float32)



# revision 17
# speedup vs baseline: 1.0109x; 1.0109x over previous
"""Block-sparse linear (x @ W.T + bias) on 8 TRN2 NeuronCores.

Data-parallel over tokens: each core computes all 4096 output features
for its 1024-token shard.  The sparse blocks are scattered into a dense
W on the host; the device runs a dense bf16 matmul (fp32 PSUM
accumulation — fp32 matmul is 4 cycles/row on the PE, bf16 is 1):

  * x shard host-converted to bf16 and transposed to SBUF layout
    [128 part, 32 kchunk, 1024 tok] (in-features on partitions),
    resident in SBUF (8 MB).
  * W.T streamed as 32 output-tiles of [128 in, 32 kchunk, 128 out]
    (1 MB each, multi-buffered DMA prefetch).
  * Per (o-tile, token half): one PSUM bank accumulates 32 K-chunk
    matmuls (K=128, M=128, N=512, bf16 in / fp32 acc), then DVE-drains
    to SBUF and DMAs to the transposed output outT [4096, 1024] fp32.

Host concatenates/transposes per-core outT shards and adds bias.
"""

import os
import sys
import time
import types
from contextlib import ExitStack

import ml_dtypes
import numpy as np

import concourse.tile as tile
from concourse import bacc, mybir
from concourse.bass_utils import run_bass_kernel_spmd


def _install_ntff_hook_shim():
    """Provide antenv.axon_hooks if the image lacks it, so BASS_TRACE=1
    can capture NTFF profiles through libaxon_pjrt."""
    try:
        import antenv.axon_hooks  # noqa: F401

        return
    except ImportError:
        pass
    try:
        import antenv
    except ImportError:
        return
    mod = types.ModuleType("antenv.axon_hooks")
    holder = {"hook": None}
    mod.set_axon_ntff_profile_hook = lambda h: holder.__setitem__("hook", h)
    mod.get_axon_ntff_profile_hook = lambda: holder["hook"]
    sys.modules["antenv.axon_hooks"] = mod
    antenv.axon_hooks = mod
    try:
        from trn_agent_boot.trn_boot import _ntff_profile_via_ctypes

        so_path = "/opt/axon/libaxon_pjrt.so"
        if os.path.exists(so_path):
            hook = _ntff_profile_via_ctypes(so_path)
            if hook is not None:
                mod.set_axon_ntff_profile_hook(hook)
    except Exception:
        pass


_install_ntff_hook_shim()

BLOCK = 32
IN_FEATURES = 4096
OUT_FEATURES = 4096
N_TOKENS = 8192
N_CORES = 8
NBC = IN_FEATURES // BLOCK  # 128
TOK_PER_CORE = N_TOKENS // N_CORES  # 1024
MM_N = 512
NPHASE = TOK_PER_CORE // MM_N  # 2
KCH = IN_FEATURES // 128  # 32 contraction chunks
OT = OUT_FEATURES // 128  # 32 output tiles
F32 = mybir.dt.float32
BF16 = mybir.dt.bfloat16
NP_BF16 = np.dtype(ml_dtypes.bfloat16)

LAST_RUN = None  # BassKernelResults of the most recent kernel() call


def _build_program():
    nc = bacc.Bacc(trn_type="TRN2", target_bir_lowering=False, debug=False)
    # x pre-split by token phase on host: [phase, 128 part, kch, 512 tok]
    xTd = nc.dram_tensor("xT", [NPHASE, 128, KCH, MM_N], BF16, kind="ExternalInput").ap()
    wst = nc.dram_tensor("wst", [OT, 128, KCH, 128], BF16, kind="ExternalInput").ap()
    outT = nc.dram_tensor(
        "outT", [OUT_FEATURES, TOK_PER_CORE], F32, kind="ExternalOutput"
    ).ap()

    with tile.TileContext(nc) as tc, ExitStack() as ctx:
        xpool = ctx.enter_context(tc.tile_pool(name="xpool", bufs=1))
        wpool = ctx.enter_context(tc.tile_pool(name="wpool", bufs=3))
        spool = ctx.enter_context(tc.tile_pool(name="spool", bufs=4))
        ppool = ctx.enter_context(tc.tile_pool(name="ppool", bufs=8, space="PSUM"))

        # First W tile ahead of the x stream so the PE can start ASAP.
        # w1 comes AFTER the phase-0 x half: o0's chain end is gated on that
        # 4.2MB delivery, so nothing else may queue ahead of it.
        w01 = []
        w = wpool.tile([128, KCH, 128], BF16, name="wt0_0", tag="wt")
        nc.sync.dma_start(w[:], wst[0])
        w01.append(w)

        # Phase-major: only the 4.2MB phase-0 half of x gates the pipeline
        # start; the phase-1 half streams in during phase-0 compute (its
        # triggers are spread across early o-iterations below so they queue
        # behind the W prefetch, not ahead of it).
        xts = []
        for ph in range(NPHASE):
            xp = xpool.tile([128, KCH, MM_N], BF16, name=f"xt{ph}", tag=f"xt{ph}")
            if ph == 0:
                for i in range(8):
                    sl = slice(4 * i, 4 * i + 4)
                    nc.sync.dma_start(xp[:, sl, :], xTd[ph, :, sl, :])
            xts.append(xp)

        w = wpool.tile([128, KCH, 128], BF16, name="wt0_1", tag="wt")
        nc.sync.dma_start(w[:], wst[1])
        w01.append(w)

        for ph in range(NPHASE):
            xp = xts[ph]
            for o in range(OT):
                if ph == 0 and o < 2:
                    wt = w01[o]
                else:
                    wt = wpool.tile([128, KCH, 128], BF16, name=f"wt{ph}_{o}", tag="wt")
                    nc.sync.dma_start(wt[:], wst[o])
                if ph == 0 and 2 <= o < 10:
                    i = o - 2
                    sl = slice(4 * i, 4 * i + 4)
                    nc.sync.dma_start(xts[1][:, sl, :], xTd[1, :, sl, :])
                ps = ppool.tile([128, MM_N], F32, name=f"ps{ph}_{o}", tag="ps")
                for k in range(KCH):
                    nc.tensor.matmul(
                        ps[:],
                        lhsT=wt[:, k, :],
                        rhs=xp[:, k, :],
                        start=(k == 0),
                        stop=(k == KCH - 1),
                    )
                stage = spool.tile([128, MM_N], F32, name=f"st{ph}_{o}", tag="st")
                nc.vector.tensor_copy(out=stage[:], in_=ps[:])
                nc.sync.dma_start(
                    outT[128 * o : 128 * o + 128, MM_N * ph : MM_N * (ph + 1)],
                    stage[:],
                )
    nc.compile()
    return nc


def kernel(x, weight_blocks, bias, block_rows, block_cols):
    global LAST_RUN
    x = np.asarray(x, np.float32)
    weight_blocks = np.asarray(weight_blocks, np.float32)
    bias = np.asarray(bias, np.float32)
    block_rows = np.asarray(block_rows).astype(np.int64)
    block_cols = np.asarray(block_cols).astype(np.int64)

    t0 = time.time()
    # dense W from sparse blocks (vectorized scatter into block grid;
    # harness block positions are a permutation => no duplicates)
    W4 = np.zeros((OUT_FEATURES // BLOCK, NBC, BLOCK, BLOCK), np.float32)
    W4[block_rows, block_cols] = weight_blocks
    W = W4.transpose(0, 2, 1, 3).reshape(OUT_FEATURES, IN_FEATURES)
    # wst[o, kk, k, mm] = W[128o+mm, 128k+kk]
    wst = np.ascontiguousarray(
        W.reshape(OT, 128, KCH, 128).transpose(0, 3, 2, 1).astype(NP_BF16)
    )

    nc = _build_program()
    t1 = time.time()

    xbf = x.astype(NP_BF16)
    in_maps = []
    for core in range(N_CORES):
        xs = xbf[core * TOK_PER_CORE : (core + 1) * TOK_PER_CORE]
        # [ph, 128 part, kch, 512 tok]; element (ph,p,k,t) = x[512ph+t, 128k+p]
        xt = np.ascontiguousarray(
            xs.T.reshape(KCH, 128, NPHASE, MM_N).transpose(2, 1, 0, 3)
        )
        in_maps.append({"xT": xt, "wst": wst})
    t2 = time.time()
    print(f"[kernel] prep={t1 - t0:.1f}s shards={t2 - t1:.1f}s", flush=True)

    res = run_bass_kernel_spmd(nc, in_maps, list(range(N_CORES)))
    LAST_RUN = res
    print(f"[kernel] run={time.time() - t2:.1f}s", flush=True)

    out = np.concatenate([res.results[i]["outT"].T for i in range(N_CORES)], axis=0)
    out = out + bias[None, :]
    return np.ascontiguousarray(out, np.float32)

